# revision 20
# baseline (speedup 1.0000x reference)
"""Trainium2 Bass kernel for nn_Decoder_LSTM (show-attend-tell style decoder).

Sharding across 8 NeuronCores (single chip):
  - Attention (enc_proj add + relu, scores, softmax, ctx) is data-parallel over
    batch: core j owns batches 8j..8j+7.
  - LSTM gates are N-sharded: core j computes gate columns {64j..64j+63} of
    each of i,f,g,o for ALL 64 batches (ctx is AllGathered), and the cell
    update for its 64 hidden dims.  h2 slices are AllGathered back (as h2^T so
    the partition-axis concat of the AllGather directly yields h^T).
  - Output projection is V-sharded: W_out columns 3750j..3750j+3749 stay
    resident in SBUF; logits computed from the replicated h^T.
The program is identical on all cores (SPMD); all core-specific data (feature
slices, weight slices, one-hot batch selector) arrives via input tensors.
"""

import sys

for _p in ("/opt/trn_rl_repo", "/root/.axon_site/_ro/trn_rl_repo"):
    if _p not in sys.path:
        sys.path.insert(0, _p)

import numpy as np
import ml_dtypes

import concourse.bass as bass
import concourse.bacc as bacc
import concourse.mybir as mybir
import concourse.tile as tile
from concourse.bass_utils import run_bass_kernel_spmd
from concourse.masks import make_identity

F32 = mybir.dt.float32
BF16 = mybir.dt.bfloat16

NC = 8
B, P, ENC = 64, 196, 2048
DEC, ATT, E, V, T = 512, 512, 300, 30000, 21
STEPS = T - 1
BL = B // NC          # 8 local batches
DS = DEC // NC        # 64 dec dims per core
VS = V // NC          # 3750 vocab per core
G4 = 4 * DS           # 256 gate cols per core
P0, P1 = 128, P - 128  # 128 + 68 partition split of P
BP = B * P // NC      # 1568 = local (b, p) axis
AF = mybir.ActivationFunctionType
ALU = mybir.AluOpType


def build_program(steps=STEPS, use_cc=True):
    nc = bacc.Bacc("TRN2", target_bir_lowering=False, debug=False,
                   num_devices=NC)
    dt = nc.dram_tensor

    # ---- per-core external inputs ----
    feat_bf = dt("feat_bf", (13, 128, ENC), BF16, kind="ExternalInput")
    featT = dt("featT", (BL, ENC, P), F32, kind="ExternalInput")
    W_enc = dt("W_enc", (ENC, ATT), F32, kind="ExternalInput")
    W_dec = dt("W_dec", (DEC, ATT), F32, kind="ExternalInput")
    w_att = dt("w_att", (ATT, 1), F32, kind="ExternalInput")
    b_enc = dt("b_enc", (ATT, 1), F32, kind="ExternalInput")
    b_dec = dt("b_dec", (ATT, 1), F32, kind="ExternalInput")
    whhT = dt("whhT", (DEC, G4), F32, kind="ExternalInput")
    wcT = dt("wcT", (ENC, G4), BF16, kind="ExternalInput")
    embproj = dt("embproj", (steps * B, G4), F32, kind="ExternalInput")
    woutT = dt("woutT", (DEC, VS), BF16, kind="ExternalInput")
    bout = dt("bout", (1, VS), F32, kind="ExternalInput")
    hT0 = dt("hT0", (DEC, B), F32, kind="ExternalInput")
    c0_sl = dt("c0_sl", (B, DS), F32, kind="ExternalInput")
    sel = dt("sel", (B, BL), F32, kind="ExternalInput")
    ones164 = dt("ones164", (1, B), F32, kind="ExternalInput")

    out_logits = dt("out_logits", (B, steps, VS), F32, kind="ExternalOutput")
    out_alphas = dt("out_alphas", (BL, steps, P), F32, kind="ExternalOutput")

    with tile.TileContext(nc) as tc:
        with tc.tile_pool(name="res", bufs=1) as res, \
             tc.tile_pool(name="dram", bufs=2, space="DRAM") as dram, \
             tc.tile_pool(name="work1", bufs=1) as work1, \
             tc.tile_pool(name="work", bufs=2) as work, \
             tc.tile_pool(name="lgs", bufs=2) as lgs, \
             tc.tile_pool(name="ctxp", bufs=2, space="PSUM") as ctxp, \
             tc.tile_pool(name="miscp", bufs=2, space="PSUM") as miscp, \
             tc.tile_pool(name="tp", bufs=2, space="PSUM") as tp, \
             tc.tile_pool(name="lgp", bufs=2, space="PSUM") as lgp:

            # ================= resident SBUF tensors =================
            featS = res.tile([128, 13 * ENC], BF16, tag="featS")
            encT = res.tile([128, 4 * BP], F32, tag="encT")
            wout_sb = res.tile([128, 4 * VS], BF16, tag="wout_sb")
            wc_sb = res.tile([128, 16 * G4], BF16, tag="wc_sb")
            wdec_sb = res.tile([128, 4 * ATT], F32, tag="wdec_sb")
            whh_sb = res.tile([128, 4 * G4], F32, tag="whh_sb")
            wa_sb = res.tile([128, 4], F32, tag="wa_sb")
            benc_sb = res.tile([128, 4], F32, tag="benc_sb")
            bdec_sb = res.tile([128, 4], F32, tag="bdec_sb")
            hT_sb = res.tile([128, 4 * B], F32, tag="hT_sb")
            hTb_sb = res.tile([128, 4 * B], BF16, tag="hTb_sb")
            hT2_sb = res.tile([128, 4 * B], F32, tag="hT2_sb")
            c_sb = res.tile([B, DS], F32, tag="c_sb")
            sel_sb = res.tile([B, BL], F32, tag="sel_sb")
            decT_sb = res.tile([128, 4 * BL], F32, tag="decT_sb")
            bd_nat = res.tile([8, 13 * 128], F32, tag="bd_nat")
            aTbd = res.tile([128, 13 * BL], BF16, tag="aTbd")
            ctxT_sb = res.tile([128, 16 * B], BF16, tag="ctxT_sb")
            ones_sb = res.tile([1, B], F32, tag="ones_sb")
            i8f = res.tile([8, 8], F32, tag="i8f")
            i64f = res.tile([64, 64], F32, tag="i64f")

            make_identity(nc, i8f[:, :])
            make_identity(nc, i64f[:, :])

            # ================= load residents =================
            for k in range(13):
                nc.sync.dma_start(featS[:, k * ENC:(k + 1) * ENC],
                                  feat_bf[k, :, :])
            nc.vector.memset(bd_nat[:, :], 0.0)
            for k in range(4):
                nc.sync.dma_start(wout_sb[:, k * VS:(k + 1) * VS],
                                  woutT[k * 128:(k + 1) * 128, :])
                nc.sync.dma_start(wdec_sb[:, k * ATT:(k + 1) * ATT],
                                  W_dec[k * 128:(k + 1) * 128, :])
                nc.sync.dma_start(whh_sb[:, k * G4:(k + 1) * G4],
                                  whhT[k * 128:(k + 1) * 128, :])
                nc.sync.dma_start(wa_sb[:, k:k + 1], w_att[k * 128:(k + 1) * 128, :])
                nc.sync.dma_start(benc_sb[:, k:k + 1], b_enc[k * 128:(k + 1) * 128, :])
                nc.sync.dma_start(bdec_sb[:, k:k + 1], b_dec[k * 128:(k + 1) * 128, :])
                nc.sync.dma_start(hT_sb[:, k * B:(k + 1) * B],
                                  hT0[k * 128:(k + 1) * 128, :])
            for k in range(16):
                nc.sync.dma_start(wc_sb[:, k * G4:(k + 1) * G4],
                                  wcT[k * 128:(k + 1) * 128, :])
            nc.sync.dma_start(ones_sb[:, :], ones164[:, :])
            nc.sync.dma_start(c_sb[:, :], c0_sl[:, :])
            nc.sync.dma_start(sel_sb[:, :], sel[:, :])
            nc.vector.tensor_copy(hTb_sb[:, :], hT_sb[:, :])
            nc.vector.tensor_copy(hT2_sb[:, :], hT_sb[:, :])

            # ================= setup: enc_projT = (feat @ W_enc + b_enc)^T ====
            # encT layout: a-tile Mt at cols [Mt*BP + b*P : .. + P]
            # Streamed: wenc row-tile [128, ATT] and featT tile [128, P] per
            # (b, kt); psum accumulates per-Mt across kt (4 bank-tiles, reusing
            # the ctxp/lgp pools' slots so setup stays within 8 PSUM banks).
            with tc.tile_pool(name="setup", bufs=2) as sup:
                for b in range(BL):
                    ps_mt = [ctxp.tile([128, P], F32, tag="ctx", name="pse0"),
                             ctxp.tile([128, P], F32, tag="ctx", name="pse1"),
                             lgp.tile([128, P], F32, tag="lg", name="pse2"),
                             lgp.tile([128, P], F32, tag="lg", name="pse3")]
                    for k in range(16):
                        wet = sup.tile([128, ATT], F32, tag="wet")
                        nc.sync.dma_start(wet[:, :], W_enc[k * 128:(k + 1) * 128, :])
                        ft = sup.tile([128, P], F32, tag="ft")
                        nc.sync.dma_start(ft[:, :], featT[b, k * 128:(k + 1) * 128, :])
                        for Mt in range(4):
                            nc.tensor.matmul(
                                ps_mt[Mt][:, :],
                                wet[:, Mt * 128:(Mt + 1) * 128],
                                ft[:, :],
                                start=(k == 0), stop=(k == 15))
                    for Mt in range(4):
                        nc.scalar.activation(
                            encT[:, Mt * BP + b * P:Mt * BP + (b + 1) * P],
                            ps_mt[Mt][:, :], AF.Identity,
                            bias=benc_sb[:, Mt:Mt + 1], scale=1.0)

            tc.strict_bb_all_engine_barrier()
            nc.tensor.nop()

            # ================= step loop =================
            for t in range(steps):
                # --- A/B: dec_projT for local batches ---
                ps_hw = miscp.tile([64, ATT], F32, tag="misc")
                for k in range(4):
                    nc.tensor.matmul(ps_hw[:, :], hT2_sb[:, k * B:k * B + B],
                                     wdec_sb[:, k * ATT:(k + 1) * ATT],
                                     start=(k == 0), stop=(k == 3))
                hw_sb = work1.tile([64, ATT], F32, tag="hw_sb")
                nc.vector.tensor_copy(hw_sb[:, :], ps_hw[:, :])
                ps_dec = miscp.tile([8, ATT], F32, tag="misc")
                nc.tensor.matmul(ps_dec[:, :], sel_sb[:, :], hw_sb[:, :],
                                 start=True, stop=True)
                dec_sb = work1.tile([8, ATT], F32, tag="dec_sb")
                nc.vector.tensor_copy(dec_sb[:, :], ps_dec[:, :])
                for Mt in range(4):
                    psT = tp.tile([128, 64], F32, tag="tp")
                    nc.tensor.transpose(psT[:, 0:8],
                                        dec_sb[:, Mt * 128:(Mt + 1) * 128],
                                        i8f[:, :])
                    nc.vector.tensor_scalar_add(
                        decT_sb[:, Mt * BL:(Mt + 1) * BL],
                        psT[:, 0:8], bdec_sb[:, Mt:Mt + 1])

                # --- C+D fused: e = relu(encT + decT[b]) per a-tile, then
                #     scores accumulate into one psum tile at 32-part offsets.
                ps_s = miscp.tile([128, 392], F32, tag="misc")
                for Mt in range(4):
                    e_t = work.tile([128, BP], F32, tag="e_t")
                    for b in range(BL):
                        src = encT[:, Mt * BP + b * P:Mt * BP + (b + 1) * P]
                        dst = e_t[:, b * P:(b + 1) * P]
                        bias = decT_sb[:, Mt * BL + b:Mt * BL + b + 1]
                        if Mt % 2 == 1:
                            nc.vector.tensor_scalar(dst, src, bias, 0.0,
                                                    ALU.add, ALU.max)
                        else:
                            nc.scalar.activation(dst, src, AF.Relu,
                                                 bias=bias, scale=1.0)
                    for ch in range(4):
                        nc.tensor.matmul(
                            ps_s[32 * ch:32 * ch + 1, :], wa_sb[:, Mt:Mt + 1],
                            e_t[:, ch * 392:(ch + 1) * 392],
                            start=(Mt == 0), stop=(Mt == 3),
                            tile_position=(0, 32 * ch))

                # --- E/F: softmax over p per local batch ---
                ssb = work.tile([128, 392], F32, tag="ssb")
                for ch in range(4):
                    nc.vector.tensor_copy(ssb[32 * ch:32 * ch + 1, :],
                                   ps_s[32 * ch:32 * ch + 1, :])
                sc8 = work.tile([BL, P], F32, tag="sc8")
                nc.sync.dma_start(sc8[0:BL:2, :], ssb[0:128:32, 0:P])
                nc.sync.dma_start(sc8[1:BL:2, :], ssb[0:128:32, P:2 * P])
                exps = work.tile([BL, P], F32, tag="exps")
                sums = work.tile([BL, 1], F32, tag="sums")
                nc.scalar.activation(exps[:, :], sc8[:, :], AF.Exp,
                                     accum_out=sums[:, :])
                rs = work.tile([BL, 1], F32, tag="rs")
                nc.vector.reciprocal(rs[:, :], sums[:, :])
                alpha = work.tile([BL, P], F32, tag="alpha")
                nc.vector.tensor_scalar_mul(alpha[:, :], exps[:, :], rs[:, :])
                nc.sync.dma_start(out_alphas[:, t, :], alpha[:, :])

                # --- G: alpha rows -> block-diagonal (DMA: arbitrary rows) ---
                for b in range(BL):
                    nc.sync.dma_start(bd_nat[b:b + 1, b * P:(b + 1) * P],
                                      alpha[b:b + 1, :])
                for k in range(13):
                    psT = tp.tile([128, 64], F32, tag="tp")
                    nc.tensor.transpose(psT[:, 0:8],
                                        bd_nat[:, k * 128:(k + 1) * 128],
                                        i8f[:, :])
                    nc.vector.tensor_copy(aTbd[:, k * BL:(k + 1) * BL],
                                          psT[:, 0:8])

                # --- H: ctx = blockdiag(alpha)^T @ featS  -> [8, ENC] ---
                # 4 ENC-chunks run in 4 concurrent PE column groups.
                ctxn = work1.tile([BL, ENC], F32, tag="ctxn")
                ps_c = ctxp.tile([128, 512], F32, tag="ctx")
                for k in range(13):
                    for dc in range(4):
                        nc.tensor.matmul(
                            ps_c[32 * dc:32 * dc + BL, :],
                            aTbd[:, k * BL:(k + 1) * BL],
                            featS[:, k * ENC + dc * 512:k * ENC + (dc + 1) * 512],
                            start=(k == 0), stop=(k == 12),
                            tile_position=(0, 32 * dc))
                for dc in range(4):
                    nc.vector.tensor_copy(ctxn[:, dc * 512:(dc + 1) * 512],
                                          ps_c[32 * dc:32 * dc + BL, :])

                # --- I: ctx AllGather ---
                ctx_in = dram.tile([BL, ENC], F32, tag="ctx_in")
                ctx_out = dram.tile([B, ENC], F32, tag="ctx_out")
                nc.sync.dma_start(ctx_in[:, :], ctxn[:, :])
                if use_cc:
                    nc.gpsimd.collective_compute(
                        "AllGather", ALU.bypass,
                        replica_groups=[list(range(NC))],
                        ins=[ctx_in.opt()], outs=[ctx_out.opt()])
                else:
                    for jj in range(NC):
                        nc.sync.dma_start(ctx_out[BL * jj:BL * (jj + 1), :],
                                          ctx_in[:, :])
                ctxall_sb = work1.tile([B, ENC], F32, tag="ctxall")
                nc.sync.dma_start(ctxall_sb[:, :], ctx_out[:, :])

                # --- J: ctxT (bf16) ---
                for dtt in range(16):
                    psT = tp.tile([128, 64], F32, tag="tp")
                    nc.tensor.transpose(psT[:, :],
                                        ctxall_sb[:, dtt * 128:(dtt + 1) * 128],
                                        i64f[:, :])
                    nc.vector.tensor_copy(ctxT_sb[:, dtt * B:(dtt + 1) * B],
                                          psT[:, :])

                # --- K: gates slice [B, G4] ---
                ps_g = miscp.tile([B, G4], F32, tag="misc")
                for k in range(4):
                    nc.tensor.matmul(ps_g[:, :], hT2_sb[:, k * B:(k + 1) * B],
                                     whh_sb[:, k * G4:(k + 1) * G4],
                                     start=(k == 0), stop=False)
                for k in range(16):
                    nc.tensor.matmul(ps_g[:, :], ctxT_sb[:, k * B:(k + 1) * B],
                                     wc_sb[:, k * G4:(k + 1) * G4],
                                     start=False, stop=(k == 15))
                emb_t = work.tile([B, G4], F32, tag="emb_t")
                nc.sync.dma_start(emb_t[:, :], embproj[t * B:(t + 1) * B, :])
                g_sb = work.tile([B, G4], F32, tag="g_sb")
                nc.vector.tensor_tensor(g_sb[:, :], ps_g[:, :], emb_t[:, :],
                                        ALU.add)

                # --- L: LSTM cell on the DS-slice ---
                sif = work.tile([B, 2 * DS], F32, tag="sif")
                nc.scalar.activation(sif[:, :], g_sb[:, 0:2 * DS], AF.Sigmoid)
                so = work.tile([B, DS], F32, tag="so")
                nc.scalar.activation(so[:, :], g_sb[:, 3 * DS:4 * DS], AF.Sigmoid)
                tg = work.tile([B, DS], F32, tag="tg")
                nc.scalar.activation(tg[:, :], g_sb[:, 2 * DS:3 * DS], AF.Tanh)
                fc = work.tile([B, DS], F32, tag="fc")
                nc.vector.tensor_tensor(fc[:, :], sif[:, DS:2 * DS], c_sb[:, :],
                                        ALU.mult)
                ig = work.tile([B, DS], F32, tag="ig")
                nc.vector.tensor_tensor(ig[:, :], sif[:, 0:DS], tg[:, :],
                                        ALU.mult)
                nc.vector.tensor_tensor(c_sb[:, :], fc[:, :], ig[:, :], ALU.add)
                tc2 = work.tile([B, DS], F32, tag="tc2")
                nc.scalar.activation(tc2[:, :], c_sb[:, :], AF.Tanh)
                h2 = work.tile([B, DS], F32, tag="h2")
                nc.vector.tensor_tensor(h2[:, :], so[:, :], tc2[:, :], ALU.mult)

                # --- M: h2^T AllGather -> new hT ---
                psT = tp.tile([128, 64], F32, tag="tp")
                nc.tensor.transpose(psT[0:64, :], h2[:, :], i64f[:, :])
                h2T = work.tile([DS, B], F32, tag="h2T")
                nc.vector.tensor_copy(h2T[:, :], psT[0:64, :])
                h_in = dram.tile([DS, B], F32, tag="h_in")
                h_out = dram.tile([DEC, B], F32, tag="h_out")
                nc.sync.dma_start(h_in[:, :], h2T[:, :])
                if use_cc:
                    nc.gpsimd.collective_compute(
                        "AllGather", ALU.bypass,
                        replica_groups=[list(range(NC))],
                        ins=[h_in.opt()], outs=[h_out.opt()])
                else:
                    for jj in range(NC):
                        nc.sync.dma_start(h_out[DS * jj:DS * (jj + 1), :],
                                          h_in[:, :])
                for k in range(4):
                    nc.sync.dma_start(hT_sb[:, k * B:(k + 1) * B],
                                      h_out[k * 128:(k + 1) * 128, :])
                nc.vector.tensor_copy(hTb_sb[:, :], hT_sb[:, :])
                nc.vector.tensor_copy(hT2_sb[:, :], hT_sb[:, :])

                # --- N: logits chunk-wise ---
                for ch in range(8):
                    n0 = 512 * ch
                    n = min(512, VS - n0)
                    ps_l = lgp.tile([B, 512], F32, tag="lg")
                    for k in range(4):
                        nc.tensor.matmul(
                            ps_l[:, 0:n], hTb_sb[:, k * B:(k + 1) * B],
                            wout_sb[:, k * VS + n0:k * VS + n0 + n],
                            start=(k == 0), stop=False)
                    bo_t = work.tile([1, 512], F32, tag="bo_t")
                    nc.sync.dma_start(bo_t[:, 0:n], bout[:, n0:n0 + n])
                    nc.tensor.matmul(ps_l[:, 0:n], ones_sb[:, :],
                                     bo_t[:, 0:n],
                                     start=False, stop=True)
                    lg_sb = lgs.tile([B, 512], F32, tag="lg_sb")
                    nc.vector.tensor_copy(lg_sb[:, 0:n], ps_l[:, 0:n])
                    nc.sync.dma_start(out_logits[:, t, n0:n0 + n],
                                      lg_sb[:, 0:n])
    nc.compile()
    return nc


_cached = {}


def _get_program():
    if "nc" not in _cached:
        _cached["nc"] = build_program()
    return _cached["nc"]


def kernel(features, captions, caption_len, emb, W_enc, b_enc, W_dec, b_dec,
           W_att, b_att, W_h0, b_h0, W_c0, b_c0, W_ih, b_ih, W_hh, b_hh,
           W_out, b_out):
    f32 = np.float32
    features = np.asarray(features, f32)
    captions = np.asarray(captions, np.int32)
    caption_len = np.asarray(caption_len, np.int32)
    arrs = {k: np.asarray(v, f32) for k, v in dict(
        emb=emb, W_enc=W_enc, b_enc=b_enc, W_dec=W_dec, b_dec=b_dec,
        W_att=W_att, b_att=b_att, W_h0=W_h0, b_h0=b_h0, W_c0=W_c0, b_c0=b_c0,
        W_ih=W_ih, b_ih=b_ih, W_hh=W_hh, b_hh=b_hh, W_out=W_out,
        b_out=b_out).items()}

    # ---- host-side glue ----
    order = np.argsort(-caption_len, kind="stable")
    features = features[order]
    captions_s = captions[order]
    seq_length = caption_len[order] - 1

    embeds = arrs["emb"][captions_s]                    # [B, T, E]
    embflat = embeds[:, :STEPS].transpose(1, 0, 2).reshape(STEPS * B, E)
    embproj_full = (embflat @ arrs["W_ih"][:, :E].T
                    + (arrs["b_ih"] + arrs["b_hh"])[None, :]).astype(f32)
    mean_f = features.mean(axis=1)
    hT0 = (mean_f @ arrs["W_h0"] + arrs["b_h0"]).T.astype(f32).copy()
    c0 = (mean_f @ arrs["W_c0"] + arrs["b_c0"]).astype(f32)

    featT = np.ascontiguousarray(features.transpose(0, 2, 1))
    feat_bf = features.astype(ml_dtypes.bfloat16)
    feat_stack = np.zeros((NC, 13 * 128, ENC), ml_dtypes.bfloat16)
    for j in range(NC):
        feat_stack[j, :BL * P] = feat_bf[BL * j:BL * (j + 1)].reshape(BL * P, ENC)
    feat_stack = feat_stack.reshape(NC, 13, 128, ENC)
    wcT_full = np.ascontiguousarray(arrs["W_ih"][:, E:].T)  # [ENC, 4DEC]
    whhT_full = np.ascontiguousarray(arrs["W_hh"].T)        # [DEC, 4DEC]

    in_maps = []
    for j in range(NC):
        gsl = np.concatenate([np.arange(DS * j, DS * (j + 1)) + g * DEC
                              for g in range(4)])
        selm = np.zeros((B, BL), f32)
        selm[BL * j:BL * (j + 1), :] = np.eye(BL, dtype=f32)
        in_maps.append({
            "feat_bf": np.ascontiguousarray(feat_stack[j]),
            "featT": np.ascontiguousarray(featT[BL * j:BL * (j + 1)]),
            "W_enc": arrs["W_enc"],
            "W_dec": arrs["W_dec"],
            "w_att": arrs["W_att"],
            "b_enc": arrs["b_enc"].reshape(ATT, 1),
            "b_dec": arrs["b_dec"].reshape(ATT, 1),
            "whhT": np.ascontiguousarray(whhT_full[:, gsl]),
            "wcT": np.ascontiguousarray(wcT_full[:, gsl]).astype(ml_dtypes.bfloat16),
            "embproj": np.ascontiguousarray(embproj_full[:, gsl]),
            "woutT": np.ascontiguousarray(
                arrs["W_out"][:, VS * j:VS * (j + 1)]).astype(ml_dtypes.bfloat16),
            "bout": np.ascontiguousarray(
                arrs["b_out"][VS * j:VS * (j + 1)]).reshape(1, VS),
            "hT0": hT0,
            "c0_sl": np.ascontiguousarray(c0[:, DS * j:DS * (j + 1)]),
            "sel": selm,
            "ones164": np.ones((1, B), f32),
        })

    nc = _get_program()
    res = run_bass_kernel_spmd(nc, in_maps, core_ids=list(range(NC)))
    _cached["last_result"] = res

    outputs = np.concatenate(
        [res.results[j]["out_logits"] for j in range(NC)], axis=2)
    alphas = np.concatenate(
        [res.results[j]["out_alphas"] for j in range(NC)], axis=0)
    return (outputs.astype(f32), alphas.astype(f32), captions_s,
            seq_length.astype(np.int32))
